# revision 23
# baseline (speedup 1.0000x reference)
"""Trainium2 Bass kernel for nn_AttitudeController (B=2097152 drones).

Contract: kernel(**inputs) takes the FULL unsharded inputs (numpy) and
returns the FULL [B, 4] float32 output.  Internally the batch is sharded
across 8 NeuronCores; each core runs an identical NEFF on its shard.

v3 design (vs the v2 interleaved-layout baseline):
  - The host transposes the inputs to PLANAR layout ([7, B] for the seven
    needed root_state columns, [4, B] for control_target) before the
    device pass.  This (a) cuts HBM input traffic from 17.8 MB to 11.5 MB
    per core, (b) turns every ScalarE extraction into a dense 1 cyc/elem
    read instead of a strided 2 cyc/elem read, and (c) lets multi-plane
    extractions batch into single ACT instructions.
  - The device output is planar fp16 [4, SHARD]; the host transposes and
    casts to float32 (exact) after the gather.
  - All per-element scale constants are folded into the ACT extraction
    instructions (free scale slot), so the Vector engine runs only
    genuine two-tensor work at fp16 2x mode.

Math (derived from the reference):
    R_des^T R = R(q_err),  q_err = q_y(th/2)* x q_x(ph/2)* x q_z(ps/2)* x q
    angle_error = [2ab, 2ac, 0]          (a,b,c,d = q_err components)
    M[:,2]      = [2(bd+ac), 2(cd-ab), 1-2(b^2+c^2)]
    rate_error  = ang_vel - yaw_rate * M[:,2]
    out[r] = sum_k Wf[r,k] * f_k - 1,  f = (2ab, 2ac, re0, re1, re2, thrust)
Wf has +-uniform-magnitude columns for the quad-X mixer, so the final
stage folds into 4 group values G0..G3 and a sign butterfly.

The quaternion is pre-scaled by sqrt(2) during extraction so that all the
quadratic monomials (AB, AC, BD, CD, B^2, C^2) come out pre-doubled.
"""

import hashlib
import math

import numpy as np

B_TOTAL = 2097152
N_CORES = 8
SHARD = B_TOTAL // N_CORES          # 262144 rows per core
P = 128                             # SBUF partitions
COLS = SHARD // P                   # 2048 columns per partition

# --- tunables -------------------------------------------------------------
COMPUTE_DT = "float16"              # intermediate dtype on-chip
TILE_WIDTHS = [192, 832, 1024]      # graduated: small first tile = short ramp
IO_RS_BUFS = 1
IO_CT_BUFS = 2
OUT_BUFS = 1
EXT_BUFS = 2
MID_BUFS = 1
GB_BUFS = 2
DVE_BUFS = 1
SQUARE_ON_ACT = True                # BB/CC via ScalarE Square LUT
E13_ON_ACT = True                   # e13 = wa*AB via ScalarE copy-scale
MAX_WAITS = 1                       # walrus (this build) allows 1 wait/inst

_SQRT2 = float(np.float32(math.sqrt(2.0)))
_PIO2 = float(np.float32(math.pi / 2.0))

_CACHE = {}


# --------------------------------------------------------------------------
# BIR post-processing: this walrus build rejects >1 sync-wait per
# instruction; split offenders into preceding Drain instructions.
# --------------------------------------------------------------------------
_bir_patch_installed = False


def _split_waits_in_bir(bir_bytes):
    import orjson

    d = orjson.loads(bir_bytes)
    changed = False
    mods = d.get("modules", [d]) if "functions" not in d else [d]
    for mod in mods:
        for fn in mod.get("functions", []):
            for blk in fn.get("blocks", []):
                out = []
                for ins in blk.get("instructions", []):
                    si = ins.get("sync_info") or {}
                    waits = si.get("on_wait") or []
                    if len(waits) > MAX_WAITS:
                        changed = True
                        chunks = [
                            waits[i : i + MAX_WAITS]
                            for i in range(0, len(waits), MAX_WAITS)
                        ]
                        for k, ch in enumerate(chunks[:-1]):
                            pre = {
                                "name": f"{ins['name']}-wsplit{k}",
                                "opcode": "Drain",
                                "engine": ins.get("engine", "SP"),
                                "ins": [],
                                "outs": [],
                                "is_reset_sema": False,
                                "sync_info": {"on_update": [], "on_wait": ch},
                            }
                            if "debug" in ins:
                                pre["debug"] = ins["debug"]
                            out.append(pre)
                        si["on_wait"] = chunks[-1]
                        ins["sync_info"] = si
                    out.append(ins)
                blk["instructions"] = out
    if changed:
        return orjson.dumps(d)
    return bir_bytes


def _install_bir_patch():
    global _bir_patch_installed
    if _bir_patch_installed:
        return
    from concourse import bass_utils

    orig = bass_utils.compile_bir_kernel

    def patched(bir_json, tmpdir, neff_name="file.neff", **kw):
        bj = bir_json if isinstance(bir_json, (bytes, bytearray)) else bir_json.encode()
        return orig(_split_waits_in_bir(bytes(bj)), tmpdir, neff_name=neff_name, **kw)

    bass_utils.compile_bir_kernel = patched
    # bass2jax imported the symbol directly
    from concourse import bass2jax

    bass2jax.compile_bir_kernel = patched
    _bir_patch_installed = True


# --------------------------------------------------------------------------
# Parameter folding
# --------------------------------------------------------------------------
def _fold_params(mass, g, mixer, max_thrusts, gain_attitude, gain_angular_rate):
    mixer = np.asarray(mixer, np.float64)
    mt = np.asarray(max_thrusts, np.float64)
    ga = np.asarray(gain_attitude, np.float64)
    gar = np.asarray(gain_angular_rate, np.float64)
    m2 = 2.0 * mixer / mt[:, None]  # [4 rotors, 4]
    Wf = np.zeros((4, 6))
    Wf[:, 0] = -m2[:, 0] * ga[0]     # coeff of 2ab
    Wf[:, 1] = -m2[:, 1] * ga[1]     # coeff of 2ac
    Wf[:, 2] = -m2[:, 0] * gar[0]    # coeff of rate_err0
    Wf[:, 3] = -m2[:, 1] * gar[1]    # coeff of rate_err1
    Wf[:, 4] = -m2[:, 2] * gar[2]    # coeff of rate_err2
    Wf[:, 5] = m2[:, 3] * float(mass) * float(g)

    def col_mag(k):
        m = np.abs(Wf[:, k])
        if not np.allclose(m, m[0], rtol=1e-5):
            raise RuntimeError(f"mixer column {k} magnitudes not uniform: {m}")
        return float(m[0])

    wa, wa1, wr, wr1, wr2, wt = (col_mag(k) for k in range(6))
    if not np.isclose(wa, wa1, rtol=1e-5):
        raise RuntimeError("wa != wa1; single-instruction e13 invalid")
    if not np.isclose(wr, wr1, rtol=1e-5):
        raise RuntimeError("wr != wr1; single-instruction avw01 invalid")
    sA = np.sign(Wf[:, 0]).astype(int)
    sB = np.sign(Wf[:, 1]).astype(int)
    sC = np.sign(Wf[:, 4]).astype(int)
    if not (np.sign(Wf[:, 2]) == sA).all():
        raise RuntimeError("columns 0/2 sign mismatch")
    if not (np.sign(Wf[:, 3]) == sB).all():
        raise RuntimeError("columns 1/3 sign mismatch")
    if not (np.sign(Wf[:, 5]) > 0).all():
        raise RuntimeError("thrust column must be positive")
    return dict(
        wa=wa, wa1=wa1, wr=wr, wr1=wr1, wr2=wr2, wt=wt,
        sA=sA.tolist(), sB=sB.tolist(), sC=sC.tolist(), Wf=Wf,
    )


def folded_numpy(root_state, control_target, fp):
    """Numpy model of exactly what the device computes (fp32). Used by
    test.py to validate the algebra separately from the hardware."""
    q = root_state[:, 3:7].astype(np.float32)
    av = root_state[:, 10:13].astype(np.float32)
    ph = control_target[:, 0]
    th = control_target[:, 1]
    ps = control_target[:, 2]
    t = control_target[:, 3]
    c, s = np.cos(ps / 2), np.sin(ps / 2)
    W, X, Y, Z = (q[:, i] * np.float32(_SQRT2) for i in range(4))
    tw = c * W + s * Z
    tx = c * X + s * Y
    ty = c * Y - s * X
    tz = c * Z - s * W
    c, s = np.cos(ph / 2), np.sin(ph / 2)
    uw = c * tw + s * tx
    ux = c * tx - s * tw
    uy = c * ty + s * tz
    uz = c * tz - s * ty
    c, s = np.cos(th / 2), np.sin(th / 2)
    A = c * uw + s * uy
    Bq = c * ux - s * uz
    Cq = c * uy - s * uw
    D = c * uz + s * ux
    AB, AC, BD, CD = A * Bq, A * Cq, Bq * D, Cq * D
    M02 = BD + AC
    M12 = CD - AB
    Sg = Bq * Bq + Cq * Cq
    pw = ps * fp["wr"]
    pw2 = ps * fp["wr2"]
    G0 = fp["wa"] * AB + fp["wr"] * av[:, 0] - pw * M02
    G1 = fp["wa1"] * AC + fp["wr1"] * av[:, 1] - pw * M12
    G2 = fp["wr2"] * av[:, 2] - pw2 + pw2 * Sg
    G3 = fp["wt"] * t - 1.0
    out = np.empty((root_state.shape[0], 4), np.float32)
    for r in range(4):
        out[:, r] = fp["sA"][r] * G0 + fp["sB"][r] * G1 + fp["sC"][r] * G2 + G3
    return out


# --------------------------------------------------------------------------
# Host-side shard/layout helpers (shared by kernel() and test.py)
# --------------------------------------------------------------------------
RS_PLANES = [3, 4, 5, 6, 10, 11, 12]   # w x y z av0 av1 av2


def make_in_maps(root_state, control_target):
    root_state = np.asarray(root_state, np.float32)
    control_target = np.asarray(control_target, np.float32)
    in_maps = []
    for i in range(N_CORES):
        sl = slice(i * SHARD, (i + 1) * SHARD)
        rsT = np.ascontiguousarray(root_state[sl, :].T[RS_PLANES])   # [7, SHARD]
        ctT = np.ascontiguousarray(control_target[sl, :].T)          # [4, SHARD]
        in_maps.append({"rsT": rsT, "ctT": ctT})
    return in_maps


def assemble_out(outT_list):
    out = np.empty((B_TOTAL, 4), np.float32)
    for i, o in enumerate(outT_list):
        out[i * SHARD : (i + 1) * SHARD, :] = o.T   # fp16 -> f32 exact
    return out


# --------------------------------------------------------------------------
# Bass program builder
# --------------------------------------------------------------------------
def _build_nc(fp, reps=1, trace_sim=False):
    import concourse.bass as bass
    import concourse.mybir as mybir
    from concourse.tile import TileContext

    f32 = mybir.dt.float32
    cdt = getattr(mybir.dt, COMPUTE_DT)

    nc = bass.Bass()

    # const AP for the pi/2 bias used by cos-via-sin
    cbias = nc.alloc_sbuf_tensor("const-f32-pio2", [128, 1], f32)
    nc.gpsimd.memset(cbias.ap(), _PIO2)
    nc.const_aps.aps[(f32, _PIO2)] = cbias.ap()
    nc.all_engine_barrier()

    rsT = nc.declare_dram_parameter("rsT", [7, SHARD], f32, isOutput=False)
    ctT = nc.declare_dram_parameter("ctT", [4, SHARD], f32, isOutput=False)
    outT = nc.declare_dram_parameter("outT", [4, SHARD], cdt, isOutput=True)
    rsT3 = rsT.rearrange("j (p c) -> p j c", p=P)
    ctT3 = ctT.rearrange("j (p c) -> p j c", p=P)
    outT3 = outT.rearrange("j (p c) -> p j c", p=P)

    assert sum(TILE_WIDTHS) == COLS
    nt = len(TILE_WIDTHS)
    offs = [sum(TILE_WIDTHS[:i]) for i in range(nt)]

    with TileContext(nc, trace_sim=trace_sim) as tc:
        with (
            tc.tile_pool(name="iors", bufs=IO_RS_BUFS) as iors,
            tc.tile_pool(name="ioct", bufs=IO_CT_BUFS) as ioct,
            tc.tile_pool(name="outp", bufs=OUT_BUFS) as outp,
            tc.tile_pool(name="ext", bufs=EXT_BUFS) as ext,
            tc.tile_pool(name="mid", bufs=MID_BUFS) as mid,
            tc.tile_pool(name="gbp", bufs=GB_BUFS) as gbp,
            tc.tile_pool(name="dve", bufs=DVE_BUFS) as dve,
        ):
            pools = dict(iors=iors, ioct=ioct, outp=outp, ext=ext, mid=mid,
                         gbp=gbp, dve=dve)
            # software-pipelined emission: the "pre" extraction of tile ti
            # (what Vector needs first: trig + q4) is emitted before the
            # body of tile ti-1, and the "post" extraction after it, so
            # the Scalar engine stays a tile ahead of Vector while the
            # body's mid-tile ACT ops (Square/e13) are not stuck behind a
            # full extraction block.
            work = [(rep, k) for rep in range(reps) for k in range(nt)]
            state = {}
            for wi in range(len(work)):
                rep, k = work[wi]
                ti = rep * nt + k
                state[ti] = _emit_load_pre(
                    nc, mybir, pools, rsT3, ctT3, ti, offs[k],
                    TILE_WIDTHS[k], fp, cdt)
                if wi == 0:
                    _emit_post(nc, mybir, pools, ti, fp, cdt, state[ti])
                    continue
                prep, pk = work[wi - 1]
                pti = prep * nt + pk
                _emit_body(nc, mybir, pools, outT3, pti, offs[pk],
                           TILE_WIDTHS[pk], fp, cdt, state.pop(pti))
                _emit_post(nc, mybir, pools, ti, fp, cdt, state[ti])
            rep, k = work[-1]
            ti = rep * nt + k
            _emit_body(nc, mybir, pools, outT3, ti, offs[k],
                       TILE_WIDTHS[k], fp, cdt, state.pop(ti))
    return nc


def _emit_load_pre(nc, mybir, pools, rsT3, ctT3, ti, c0, Cw, fp, cdt):
    """DMA loads + the ScalarE extractions Vector needs first (trig, q4)."""
    f32 = mybir.dt.float32
    AF = mybir.ActivationFunctionType

    # ct first: it gates the trig extraction, which gates Vector.  The rs
    # q-planes DMA follows; the av-planes DMA is deferred to "post" (its
    # consumer runs a tile later).  All ct readers live in "pre" so with
    # single-buffered io the next ct DMA is released early.
    ct_t = pools["ioct"].tile([P, 4 * Cw], f32, tag="ct", name=f"ct_{ti}")
    ct3 = ct_t.rearrange("p (j c) -> p j c", c=Cw)
    nc.sync.dma_start(out=ct3[:], in_=ctT3[:, :, c0 : c0 + Cw])
    rsq_t = pools["iors"].tile([P, 4 * Cw], f32, tag="rsq", name=f"rsq_{ti}")
    rsq = rsq_t.rearrange("p (j c) -> p j c", c=Cw)
    nc.sync.dma_start(out=rsq[:], in_=rsT3[:, 0:4, c0 : c0 + Cw])

    def etile(name, k):
        return pools["ext"].tile([P, k * Cw], cdt, tag=name, name=f"{name}_{ti}")

    def v(ap, k):
        return ap.rearrange("p (k c) -> p k c", c=Cw)

    ACT = nc.scalar.activation

    # trig planes: sc = sin(angle/2), cc = cos(angle/2)  (roll,pitch,yaw)
    sc = etile("sc", 3)
    ACT(v(sc, 3), ct3[:, 0:3, :], AF.Sin, scale=0.5)
    cc = etile("cc", 3)
    ACT(v(cc, 3), ct3[:, 0:3, :], AF.Sin, scale=0.5, bias=_PIO2)
    # q4 = sqrt(2) * (w, x, y, z): planar fp16
    q4 = etile("q4", 4)
    ACT(v(q4, 4), rsq[:, :, :], AF.Copy, scale=_SQRT2)
    # GB plane 0 = G3 = wt*T - 1
    GB = pools["gbp"].tile([P, 4 * Cw], cdt, tag="GB", name=f"GB_{ti}")
    GBv = v(GB, 4)
    ACT(GBv[:, 0, :], ct3[:, 3, :], AF.Copy, scale=fp["wt"], bias=-1.0)
    # psw = wr * psi ; pv2 = wr2 * psi
    ps2 = etile("ps2", 2)
    ps2v = v(ps2, 2)
    ACT(ps2v[:, 0, :], ct3[:, 2, :], AF.Copy, scale=fp["wr"])
    ACT(ps2v[:, 1, :], ct3[:, 2, :], AF.Copy, scale=fp["wr2"])
    return dict(q4=q4, sc=sc, cc=cc, GB=GB, ps2=ps2, rsT3=rsT3, c0=c0,
                Cw=Cw, ti=ti)


def _emit_post(nc, mybir, pools, ti, fp, cdt, st):
    """av-planes load + extraction (consumed late in the body)."""
    f32 = mybir.dt.float32
    AF = mybir.ActivationFunctionType
    Cw, c0, rsT3 = st["Cw"], st["c0"], st["rsT3"]

    rsa_t = pools["iors"].tile([P, 3 * Cw], f32, tag="rsa", name=f"rsa_{ti}")
    rsa = rsa_t.rearrange("p (j c) -> p j c", c=Cw)
    nc.sync.dma_start(out=rsa[:], in_=rsT3[:, 4:7, c0 : c0 + Cw])

    def v(ap, k):
        return ap.rearrange("p (k c) -> p k c", c=Cw)

    ACT = nc.scalar.activation

    # avw01 = wr * (av0, av1);  av2w = wr2 * av2
    avw = pools["ext"].tile([P, 3 * Cw], cdt, tag="avw", name=f"avw_{ti}")
    avwv = v(avw, 3)
    ACT(avwv[:, 0:2, :], rsa[:, 0:2, :], AF.Copy, scale=fp["wr"])
    ACT(avwv[:, 2, :], rsa[:, 2, :], AF.Copy, scale=fp["wr2"])
    st.update(avw=avw)


def _emit_body(nc, mybir, pools, outT3, ti, c0, Cw, fp, cdt, st):
    """Vector-engine body (+ mid-tile ACT Square/e13) for one tile."""
    AF = mybir.ActivationFunctionType
    OP = mybir.AluOpType
    TT = nc.vector.tensor_tensor
    ACT = nc.scalar.activation

    q4, avw, sc, cc, GB, ps2 = (st[k] for k in ("q4", "avw", "sc", "cc",
                                                "GB", "ps2"))

    def v(ap, k):
        return ap.rearrange("p (k c) -> p k c", c=Cw)

    q4v, avwv, scv, ccv, GBv, ps2v = (v(x, k) for x, k in
                                      ((q4, 4), (avw, 3), (sc, 3), (cc, 3),
                                       (GB, 4), (ps2, 2)))

    # ---- temp allocator with per-width tag free lists ----
    free_tags = {}
    n_tags = [0]
    tag_of = {}

    def alloc(name, k=1, pool="dve"):
        fl = free_tags.setdefault((pool, k), [])
        if fl:
            tag = fl.pop()
        else:
            tag = f"{pool}w{k}_{n_tags[0]}"
            n_tags[0] += 1
        ap = pools[pool].tile([P, k * Cw], cdt, tag=tag, name=f"{name}_{ti}")
        tag_of[id(ap)] = (pool, tag, k)
        return ap

    def freet(*aps):
        for ap in aps:
            pool, tag, k = tag_of.pop(id(ap))
            free_tags[(pool, k)].append(tag)

    def bc(ap_pc, k):
        """broadcast a [P, Cw] AP across k components -> [P, k, Cw]"""
        return (ap_pc.rearrange("p (k c) -> p k c", k=1)
                .to_broadcast([P, k, Cw]))

    def bc4d(ap_pc):
        return (ap_pc.rearrange("p (a b c) -> p a b c", a=1, b=1)
                .to_broadcast([P, 2, 2, Cw]))

    # ---- stage 1: q_z* x q   (angle idx 2 = psi; pairs (W,Z),(X,Y)) ----
    mc = alloc("mc", 4)
    ms = alloc("ms", 4)
    mcv, msv = v(mc, 4), v(ms, 4)
    TT(mcv[:, :], bc(ccv[:, 2, :], 4), q4v[:, :], OP.mult)
    TT(msv[:, :], bc(scv[:, 2, :], 4), q4v[:, ::-1], OP.mult)
    t4 = alloc("t4", 4)
    t4v = v(t4, 4)
    TT(t4v[:, 0:2], mcv[:, 0:2], msv[:, 0:2], OP.add)
    TT(t4v[:, 2:4], mcv[:, 2:4], msv[:, 2:4], OP.subtract)

    # ---- stage 2: q_x* x t  (angle idx 0 = roll; swap within pairs) ----
    TT(mcv[:, :], bc(ccv[:, 0, :], 4), t4v[:, :], OP.mult)
    ms4d = ms.rearrange("p (a b c) -> p a b c", a=2, c=Cw)
    t4sw = t4.rearrange("p (a b c) -> p a b c", a=2, c=Cw)[:, :, ::-1]
    TT(ms4d, bc4d(scv[:, 0, :]), t4sw, OP.mult)
    freet(t4)
    u4 = alloc("u4", 4)      # aliases t4's slot (t4 fully consumed above)
    u4v = v(u4, 4)
    TT(u4v[:, 0:4:2], mcv[:, 0:4:2], msv[:, 0:4:2], OP.add)
    TT(u4v[:, 1:4:2], mcv[:, 1:4:2], msv[:, 1:4:2], OP.subtract)

    # ---- stage 3: q_y* x u  (angle idx 1 = pitch; rotate-2) ----
    TT(mcv[:, :], bc(ccv[:, 1, :], 4), u4v[:, :], OP.mult)
    ms4r = ms.rearrange("p (a b c) -> p a b c", b=2, c=Cw)
    u4rot = u4.rearrange("p (a b c) -> p a b c", b=2, c=Cw)[:, ::-1]
    TT(ms4r, bc4d(scv[:, 1, :]), u4rot, OP.mult)
    freet(u4)
    a4 = alloc("a4", 4)      # aliases u4's slot
    a4v = v(a4, 4)
    TT(a4v[:, 0:4:3], mcv[:, 0:4:3], msv[:, 0:4:3], OP.add)
    TT(a4v[:, 1:3], mcv[:, 1:3], msv[:, 1:3], OP.subtract)
    freet(mc, ms)

    # ---- products: Pab = (AB, AC), Pcd = (BD, CD), BBCC via ACT ----
    Pab = alloc("Pab", 2)
    Pabv = v(Pab, 2)
    TT(Pabv[:, :], bc(a4v[:, 0], 2), a4v[:, 1:3], OP.mult)
    Pcd = alloc("Pcd", 2)
    Pcdv = v(Pcd, 2)
    TT(Pcdv[:, :], a4v[:, 1:3], bc(a4v[:, 3], 2), OP.mult)
    if SQUARE_ON_ACT:
        bbcc = pools["mid"].tile([P, 2 * Cw], cdt, tag="bbcc",
                                 name=f"bbcc_{ti}")
        ACT(v(bbcc, 2), a4v[:, 1:3], AF.Square)
    else:
        bbcc = alloc("bbcc", 2)
        TT(v(bbcc, 2), a4v[:, 1:3], a4v[:, 1:3], OP.mult)
    if E13_ON_ACT:
        e13 = pools["mid"].tile([P, 2 * Cw], cdt, tag="e13", name=f"e13_{ti}")
        ACT(v(e13, 2), Pabv[:, :], AF.Copy, scale=fp["wa"])
    else:
        e13 = None
    freet(a4)

    # ---- M2 = (M02, M12) ----
    M2 = alloc("M2", 2)
    M2v = v(M2, 2)
    TT(M2v[:, 0], Pcdv[:, 0], Pabv[:, 1], OP.add)
    TT(M2v[:, 1], Pcdv[:, 1], Pabv[:, 0], OP.subtract)
    if E13_ON_ACT:
        freet(Pab, Pcd)   # e13 (ACT) was their last reader
    # s01 = (wr*psi) * M2
    s01 = alloc("s01", 2)
    TT(v(s01, 2)[:, :], bc(ps2v[:, 0, :], 2), M2v[:, :], OP.mult)
    freet(M2)
    # Sg = BB + CC ; s2 = (wr2*psi) * Sg
    Sg = alloc("Sg")
    TT(Sg[:], v(bbcc, 2)[:, 0], v(bbcc, 2)[:, 1], OP.add)
    s2 = alloc("s2")
    TT(s2[:], ps2v[:, 1, :], Sg[:], OP.mult)
    freet(Sg)

    # ---- G values into GB = (G3, G1, G0, G2) ----
    t01 = alloc("t01", 2)
    if e13 is None:
        e13l = alloc("e13", 2)
        nc.vector.tensor_scalar_mul(v(e13l, 2)[:, :], Pabv[:, :], fp["wa"])
        TT(v(t01, 2)[:, :], v(e13l, 2)[:, :], avwv[:, 0:2, :], OP.add)
        freet(e13l)
    else:
        TT(v(t01, 2)[:, :], v(e13, 2)[:, :], avwv[:, 0:2, :], OP.add)
    if not E13_ON_ACT:
        freet(Pab, Pcd)
    TT(GBv[:, 2:0:-1], v(t01, 2)[:, :], v(s01, 2)[:, :], OP.subtract)
    freet(t01, s01)
    g2a = alloc("g2a")
    TT(g2a[:], avwv[:, 2, :], ps2v[:, 1, :], OP.subtract)
    TT(GBv[:, 3], g2a[:], s2[:], OP.add)
    freet(g2a, s2)

    # ---- butterfly: UV = (U+, V+, U-, V-) ----
    UV = alloc("UV", 4)
    UVv = v(UV, 4)
    TT(UVv[:, 0:2], GBv[:, 0:2], GBv[:, 2:4], OP.add)
    TT(UVv[:, 2:4], GBv[:, 0:2], GBv[:, 2:4], OP.subtract)

    # ---- outs: out[r] = U_{sA[r]} + sB[r] * V_{sB[r]*sC[r]} ----
    out_t = pools["outp"].tile([P, 4 * Cw], cdt, tag="out", name=f"out_{ti}")
    out3 = v(out_t, 4)
    uidx = [0 if fp["sA"][r] > 0 else 2 for r in range(4)]
    vidx = [1 if fp["sB"][r] * fp["sC"][r] > 0 else 3 for r in range(4)]
    adds = [r for r in range(4) if fp["sB"][r] > 0]
    subs = [r for r in range(4) if fp["sB"][r] <= 0]

    def emit_outs(rset, op):
        # each computed group is DMA'd out immediately so the final
        # store overlaps the remaining Vector work
        while rset:
            if len(rset) >= 2:
                a, b = rset[0], rset[1]
                dst = out3[:, a : b + 1 : (b - a), :]

                def pair_ap(ia, ib):
                    if ib == ia:
                        return bc(UVv[:, ia], 2)
                    if ib > ia:
                        return UVv[:, ia : ib + 1 : ib - ia]
                    return UVv[:, ia :: ib - ia]

                TT(dst, pair_ap(uidx[a], uidx[b]), pair_ap(vidx[a], vidx[b]),
                   op)
                nc.sync.dma_start(
                    out=outT3[:, a : b + 1 : (b - a), c0 : c0 + Cw], in_=dst)
                rset = rset[2:]
            else:
                r = rset[0]
                TT(out3[:, r, :], UVv[:, uidx[r]], UVv[:, vidx[r]], op)
                nc.sync.dma_start(
                    out=outT3[:, r, c0 : c0 + Cw], in_=out3[:, r, :])
                rset = rset[1:]

    emit_outs(adds, OP.add)
    emit_outs(subs, OP.subtract)
    freet(UV)


# --------------------------------------------------------------------------
# Public entry point
# --------------------------------------------------------------------------
def kernel(root_state, control_target, mass, g, mixer, max_thrusts,
           gain_attitude, gain_angular_rate):
    root_state = np.asarray(root_state, np.float32)
    control_target = np.asarray(control_target, np.float32)
    assert root_state.shape == (B_TOTAL, 13), root_state.shape
    assert control_target.shape == (B_TOTAL, 4), control_target.shape

    fp = _fold_params(mass, g, mixer, max_thrusts, gain_attitude, gain_angular_rate)

    key = hashlib.sha256(
        repr(({k: v for k, v in fp.items() if k != "Wf"}, COMPUTE_DT,
              tuple(TILE_WIDTHS), SQUARE_ON_ACT, E13_ON_ACT)).encode()
    ).hexdigest()
    if key not in _CACHE:
        _install_bir_patch()
        _CACHE[key] = _build_nc(fp)
    nc = _CACHE[key]

    from concourse.bass_utils import run_bass_kernel_spmd

    in_maps = make_in_maps(root_state, control_target)
    res = run_bass_kernel_spmd(nc, in_maps, core_ids=list(range(N_CORES)))
    return assemble_out([res.results[i]["outT"] for i in range(N_CORES)])


# revision 24
# speedup vs baseline: 1.0889x; 1.0889x over previous
"""Trainium2 Bass kernel for nn_AttitudeController (B=2097152 drones).

Contract: kernel(**inputs) takes the FULL unsharded inputs (numpy) and
returns the FULL [B, 4] float32 output.  Internally the batch is sharded
across 8 NeuronCores; each core runs an identical NEFF on its shard.

v3 design (vs the v2 interleaved-layout baseline):
  - The host transposes the inputs to PLANAR layout ([7, B] for the seven
    needed root_state columns, [4, B] for control_target) before the
    device pass.  This (a) cuts HBM input traffic from 17.8 MB to 11.5 MB
    per core, (b) turns every ScalarE extraction into a dense 1 cyc/elem
    read instead of a strided 2 cyc/elem read, and (c) lets multi-plane
    extractions batch into single ACT instructions.
  - The device output is planar fp16 [4, SHARD]; the host transposes and
    casts to float32 (exact) after the gather.
  - All per-element scale constants are folded into the ACT extraction
    instructions (free scale slot), so the Vector engine runs only
    genuine two-tensor work at fp16 2x mode.

Math (derived from the reference):
    R_des^T R = R(q_err),  q_err = q_y(th/2)* x q_x(ph/2)* x q_z(ps/2)* x q
    angle_error = [2ab, 2ac, 0]          (a,b,c,d = q_err components)
    M[:,2]      = [2(bd+ac), 2(cd-ab), 1-2(b^2+c^2)]
    rate_error  = ang_vel - yaw_rate * M[:,2]
    out[r] = sum_k Wf[r,k] * f_k - 1,  f = (2ab, 2ac, re0, re1, re2, thrust)
Wf has +-uniform-magnitude columns for the quad-X mixer, so the final
stage folds into 4 group values G0..G3 and a sign butterfly.

The quaternion is pre-scaled by sqrt(2) during extraction so that all the
quadratic monomials (AB, AC, BD, CD, B^2, C^2) come out pre-doubled.
"""

import hashlib
import math

import numpy as np

B_TOTAL = 2097152
N_CORES = 8
SHARD = B_TOTAL // N_CORES          # 262144 rows per core
P = 128                             # SBUF partitions
COLS = SHARD // P                   # 2048 columns per partition

# --- tunables -------------------------------------------------------------
COMPUTE_DT = "float16"              # intermediate dtype on-chip
TILE_WIDTHS = [192, 352, 608, 896]  # geometric ramp: DMA/ACT stay ahead of V
IO_RS_BUFS = 1
IO_CT_BUFS = 2
OUT_BUFS = 2
EXT_BUFS = 2
MID_BUFS = 2
GB_BUFS = 2
DVE_BUFS = 1
SQUARE_ON_ACT = True                # BB/CC via ScalarE Square LUT
E13_ON_ACT = True                   # e13 = wa*AB via ScalarE copy-scale
MAX_WAITS = 1                       # walrus (this build) allows 1 wait/inst

_SQRT2 = float(np.float32(math.sqrt(2.0)))
_PIO2 = float(np.float32(math.pi / 2.0))

_CACHE = {}


# --------------------------------------------------------------------------
# BIR post-processing: this walrus build rejects >1 sync-wait per
# instruction; split offenders into preceding Drain instructions.
# --------------------------------------------------------------------------
_bir_patch_installed = False


def _split_waits_in_bir(bir_bytes):
    import orjson

    d = orjson.loads(bir_bytes)
    changed = False
    mods = d.get("modules", [d]) if "functions" not in d else [d]
    for mod in mods:
        for fn in mod.get("functions", []):
            for blk in fn.get("blocks", []):
                out = []
                for ins in blk.get("instructions", []):
                    si = ins.get("sync_info") or {}
                    waits = si.get("on_wait") or []
                    if len(waits) > MAX_WAITS:
                        changed = True
                        chunks = [
                            waits[i : i + MAX_WAITS]
                            for i in range(0, len(waits), MAX_WAITS)
                        ]
                        for k, ch in enumerate(chunks[:-1]):
                            pre = {
                                "name": f"{ins['name']}-wsplit{k}",
                                "opcode": "Drain",
                                "engine": ins.get("engine", "SP"),
                                "ins": [],
                                "outs": [],
                                "is_reset_sema": False,
                                "sync_info": {"on_update": [], "on_wait": ch},
                            }
                            if "debug" in ins:
                                pre["debug"] = ins["debug"]
                            out.append(pre)
                        si["on_wait"] = chunks[-1]
                        ins["sync_info"] = si
                    out.append(ins)
                blk["instructions"] = out
    if changed:
        return orjson.dumps(d)
    return bir_bytes


def _install_bir_patch():
    global _bir_patch_installed
    if _bir_patch_installed:
        return
    from concourse import bass_utils

    orig = bass_utils.compile_bir_kernel

    def patched(bir_json, tmpdir, neff_name="file.neff", **kw):
        bj = bir_json if isinstance(bir_json, (bytes, bytearray)) else bir_json.encode()
        return orig(_split_waits_in_bir(bytes(bj)), tmpdir, neff_name=neff_name, **kw)

    bass_utils.compile_bir_kernel = patched
    # bass2jax imported the symbol directly
    from concourse import bass2jax

    bass2jax.compile_bir_kernel = patched
    _bir_patch_installed = True


# --------------------------------------------------------------------------
# Parameter folding
# --------------------------------------------------------------------------
def _fold_params(mass, g, mixer, max_thrusts, gain_attitude, gain_angular_rate):
    mixer = np.asarray(mixer, np.float64)
    mt = np.asarray(max_thrusts, np.float64)
    ga = np.asarray(gain_attitude, np.float64)
    gar = np.asarray(gain_angular_rate, np.float64)
    m2 = 2.0 * mixer / mt[:, None]  # [4 rotors, 4]
    Wf = np.zeros((4, 6))
    Wf[:, 0] = -m2[:, 0] * ga[0]     # coeff of 2ab
    Wf[:, 1] = -m2[:, 1] * ga[1]     # coeff of 2ac
    Wf[:, 2] = -m2[:, 0] * gar[0]    # coeff of rate_err0
    Wf[:, 3] = -m2[:, 1] * gar[1]    # coeff of rate_err1
    Wf[:, 4] = -m2[:, 2] * gar[2]    # coeff of rate_err2
    Wf[:, 5] = m2[:, 3] * float(mass) * float(g)

    def col_mag(k):
        m = np.abs(Wf[:, k])
        if not np.allclose(m, m[0], rtol=1e-5):
            raise RuntimeError(f"mixer column {k} magnitudes not uniform: {m}")
        return float(m[0])

    wa, wa1, wr, wr1, wr2, wt = (col_mag(k) for k in range(6))
    if not np.isclose(wa, wa1, rtol=1e-5):
        raise RuntimeError("wa != wa1; single-instruction e13 invalid")
    if not np.isclose(wr, wr1, rtol=1e-5):
        raise RuntimeError("wr != wr1; single-instruction avw01 invalid")
    sA = np.sign(Wf[:, 0]).astype(int)
    sB = np.sign(Wf[:, 1]).astype(int)
    sC = np.sign(Wf[:, 4]).astype(int)
    if not (np.sign(Wf[:, 2]) == sA).all():
        raise RuntimeError("columns 0/2 sign mismatch")
    if not (np.sign(Wf[:, 3]) == sB).all():
        raise RuntimeError("columns 1/3 sign mismatch")
    if not (np.sign(Wf[:, 5]) > 0).all():
        raise RuntimeError("thrust column must be positive")
    return dict(
        wa=wa, wa1=wa1, wr=wr, wr1=wr1, wr2=wr2, wt=wt,
        sA=sA.tolist(), sB=sB.tolist(), sC=sC.tolist(), Wf=Wf,
    )


def folded_numpy(root_state, control_target, fp):
    """Numpy model of exactly what the device computes (fp32). Used by
    test.py to validate the algebra separately from the hardware."""
    q = root_state[:, 3:7].astype(np.float32)
    av = root_state[:, 10:13].astype(np.float32)
    ph = control_target[:, 0]
    th = control_target[:, 1]
    ps = control_target[:, 2]
    t = control_target[:, 3]
    c, s = np.cos(ps / 2), np.sin(ps / 2)
    W, X, Y, Z = (q[:, i] * np.float32(_SQRT2) for i in range(4))
    tw = c * W + s * Z
    tx = c * X + s * Y
    ty = c * Y - s * X
    tz = c * Z - s * W
    c, s = np.cos(ph / 2), np.sin(ph / 2)
    uw = c * tw + s * tx
    ux = c * tx - s * tw
    uy = c * ty + s * tz
    uz = c * tz - s * ty
    c, s = np.cos(th / 2), np.sin(th / 2)
    A = c * uw + s * uy
    Bq = c * ux - s * uz
    Cq = c * uy - s * uw
    D = c * uz + s * ux
    AB, AC, BD, CD = A * Bq, A * Cq, Bq * D, Cq * D
    M02 = BD + AC
    M12 = CD - AB
    Sg = Bq * Bq + Cq * Cq
    pw = ps * fp["wr"]
    pw2 = ps * fp["wr2"]
    G0 = fp["wa"] * AB + fp["wr"] * av[:, 0] - pw * M02
    G1 = fp["wa1"] * AC + fp["wr1"] * av[:, 1] - pw * M12
    G2 = fp["wr2"] * av[:, 2] - pw2 + pw2 * Sg
    G3 = fp["wt"] * t - 1.0
    out = np.empty((root_state.shape[0], 4), np.float32)
    for r in range(4):
        out[:, r] = fp["sA"][r] * G0 + fp["sB"][r] * G1 + fp["sC"][r] * G2 + G3
    return out


# --------------------------------------------------------------------------
# Host-side shard/layout helpers (shared by kernel() and test.py)
# --------------------------------------------------------------------------
RS_PLANES = [3, 4, 5, 6, 10, 11, 12]   # w x y z av0 av1 av2


def make_in_maps(root_state, control_target):
    root_state = np.asarray(root_state, np.float32)
    control_target = np.asarray(control_target, np.float32)
    in_maps = []
    for i in range(N_CORES):
        sl = slice(i * SHARD, (i + 1) * SHARD)
        rsT = np.ascontiguousarray(root_state[sl, :].T[RS_PLANES])   # [7, SHARD]
        ctT = np.ascontiguousarray(control_target[sl, :].T)          # [4, SHARD]
        in_maps.append({"rsT": rsT, "ctT": ctT})
    return in_maps


def assemble_out(outT_list):
    out = np.empty((B_TOTAL, 4), np.float32)
    for i, o in enumerate(outT_list):
        out[i * SHARD : (i + 1) * SHARD, :] = o.T   # fp16 -> f32 exact
    return out


# --------------------------------------------------------------------------
# Bass program builder
# --------------------------------------------------------------------------
def _build_nc(fp, reps=1, trace_sim=False):
    import concourse.bass as bass
    import concourse.mybir as mybir
    from concourse.tile import TileContext

    f32 = mybir.dt.float32
    cdt = getattr(mybir.dt, COMPUTE_DT)

    nc = bass.Bass()

    # const AP for the pi/2 bias used by cos-via-sin
    cbias = nc.alloc_sbuf_tensor("const-f32-pio2", [128, 1], f32)
    nc.gpsimd.memset(cbias.ap(), _PIO2)
    nc.const_aps.aps[(f32, _PIO2)] = cbias.ap()
    nc.all_engine_barrier()

    rsT = nc.declare_dram_parameter("rsT", [7, SHARD], f32, isOutput=False)
    ctT = nc.declare_dram_parameter("ctT", [4, SHARD], f32, isOutput=False)
    outT = nc.declare_dram_parameter("outT", [4, SHARD], cdt, isOutput=True)
    rsT3 = rsT.rearrange("j (p c) -> p j c", p=P)
    ctT3 = ctT.rearrange("j (p c) -> p j c", p=P)
    outT3 = outT.rearrange("j (p c) -> p j c", p=P)

    assert sum(TILE_WIDTHS) == COLS
    nt = len(TILE_WIDTHS)
    offs = [sum(TILE_WIDTHS[:i]) for i in range(nt)]

    with TileContext(nc, trace_sim=trace_sim) as tc:
        with (
            tc.tile_pool(name="iors", bufs=IO_RS_BUFS) as iors,
            tc.tile_pool(name="ioct", bufs=IO_CT_BUFS) as ioct,
            tc.tile_pool(name="outp", bufs=OUT_BUFS) as outp,
            tc.tile_pool(name="ext", bufs=EXT_BUFS) as ext,
            tc.tile_pool(name="mid", bufs=MID_BUFS) as mid,
            tc.tile_pool(name="gbp", bufs=GB_BUFS) as gbp,
            tc.tile_pool(name="dve", bufs=DVE_BUFS) as dve,
        ):
            pools = dict(iors=iors, ioct=ioct, outp=outp, ext=ext, mid=mid,
                         gbp=gbp, dve=dve)
            # software-pipelined emission: the "pre" extraction of tile ti
            # (what Vector needs first: trig + q4) is emitted before the
            # body of tile ti-1, and the "post" extraction after it, so
            # the Scalar engine stays a tile ahead of Vector while the
            # body's mid-tile ACT ops (Square/e13) are not stuck behind a
            # full extraction block.
            work = [(rep, k) for rep in range(reps) for k in range(nt)]
            state = {}
            for wi in range(len(work)):
                rep, k = work[wi]
                ti = rep * nt + k
                state[ti] = _emit_load_pre(
                    nc, mybir, pools, rsT3, ctT3, ti, offs[k],
                    TILE_WIDTHS[k], fp, cdt)
                if wi == 0:
                    _emit_post(nc, mybir, pools, ti, fp, cdt, state[ti])
                    continue
                prep, pk = work[wi - 1]
                pti = prep * nt + pk
                _emit_body(nc, mybir, pools, outT3, pti, offs[pk],
                           TILE_WIDTHS[pk], fp, cdt, state.pop(pti))
                _emit_post(nc, mybir, pools, ti, fp, cdt, state[ti])
            rep, k = work[-1]
            ti = rep * nt + k
            _emit_body(nc, mybir, pools, outT3, ti, offs[k],
                       TILE_WIDTHS[k], fp, cdt, state.pop(ti))
    return nc


def _emit_load_pre(nc, mybir, pools, rsT3, ctT3, ti, c0, Cw, fp, cdt):
    """DMA loads + the ScalarE extractions Vector needs first (trig, q4)."""
    f32 = mybir.dt.float32
    AF = mybir.ActivationFunctionType

    # ct first: it gates the trig extraction, which gates Vector.  The rs
    # q-planes DMA follows; the av-planes DMA is deferred to "post" (its
    # consumer runs a tile later).  All ct readers live in "pre" so with
    # single-buffered io the next ct DMA is released early.
    ct_t = pools["ioct"].tile([P, 4 * Cw], f32, tag="ct", name=f"ct_{ti}")
    ct3 = ct_t.rearrange("p (j c) -> p j c", c=Cw)
    nc.sync.dma_start(out=ct3[:], in_=ctT3[:, :, c0 : c0 + Cw])
    rsq_t = pools["iors"].tile([P, 4 * Cw], f32, tag="rsq", name=f"rsq_{ti}")
    rsq = rsq_t.rearrange("p (j c) -> p j c", c=Cw)
    nc.sync.dma_start(out=rsq[:], in_=rsT3[:, 0:4, c0 : c0 + Cw])

    def etile(name, k):
        return pools["ext"].tile([P, k * Cw], cdt, tag=name, name=f"{name}_{ti}")

    def v(ap, k):
        return ap.rearrange("p (k c) -> p k c", c=Cw)

    ACT = nc.scalar.activation

    # trig planes: sc = sin(angle/2), cc = cos(angle/2)  (roll,pitch,yaw)
    sc = etile("sc", 3)
    ACT(v(sc, 3), ct3[:, 0:3, :], AF.Sin, scale=0.5)
    cc = etile("cc", 3)
    ACT(v(cc, 3), ct3[:, 0:3, :], AF.Sin, scale=0.5, bias=_PIO2)
    # q4 = sqrt(2) * (w, x, y, z): planar fp16
    q4 = etile("q4", 4)
    ACT(v(q4, 4), rsq[:, :, :], AF.Copy, scale=_SQRT2)
    # GB plane 0 = G3 = wt*T - 1
    GB = pools["gbp"].tile([P, 4 * Cw], cdt, tag="GB", name=f"GB_{ti}")
    GBv = v(GB, 4)
    ACT(GBv[:, 0, :], ct3[:, 3, :], AF.Copy, scale=fp["wt"], bias=-1.0)
    # psw = wr * psi ; pv2 = wr2 * psi
    ps2 = etile("ps2", 2)
    ps2v = v(ps2, 2)
    ACT(ps2v[:, 0, :], ct3[:, 2, :], AF.Copy, scale=fp["wr"])
    ACT(ps2v[:, 1, :], ct3[:, 2, :], AF.Copy, scale=fp["wr2"])
    return dict(q4=q4, sc=sc, cc=cc, GB=GB, ps2=ps2, rsT3=rsT3, c0=c0,
                Cw=Cw, ti=ti)


def _emit_post(nc, mybir, pools, ti, fp, cdt, st):
    """av-planes load + extraction (consumed late in the body)."""
    f32 = mybir.dt.float32
    AF = mybir.ActivationFunctionType
    Cw, c0, rsT3 = st["Cw"], st["c0"], st["rsT3"]

    rsa_t = pools["iors"].tile([P, 3 * Cw], f32, tag="rsa", name=f"rsa_{ti}")
    rsa = rsa_t.rearrange("p (j c) -> p j c", c=Cw)
    nc.sync.dma_start(out=rsa[:], in_=rsT3[:, 4:7, c0 : c0 + Cw])

    def v(ap, k):
        return ap.rearrange("p (k c) -> p k c", c=Cw)

    ACT = nc.scalar.activation

    # avw01 = wr * (av0, av1);  av2w = wr2 * av2
    avw = pools["ext"].tile([P, 3 * Cw], cdt, tag="avw", name=f"avw_{ti}")
    avwv = v(avw, 3)
    ACT(avwv[:, 0:2, :], rsa[:, 0:2, :], AF.Copy, scale=fp["wr"])
    ACT(avwv[:, 2, :], rsa[:, 2, :], AF.Copy, scale=fp["wr2"])
    st.update(avw=avw)


def _emit_body(nc, mybir, pools, outT3, ti, c0, Cw, fp, cdt, st):
    """Vector-engine body (+ mid-tile ACT Square/e13) for one tile."""
    AF = mybir.ActivationFunctionType
    OP = mybir.AluOpType
    TT = nc.vector.tensor_tensor
    ACT = nc.scalar.activation

    q4, avw, sc, cc, GB, ps2 = (st[k] for k in ("q4", "avw", "sc", "cc",
                                                "GB", "ps2"))

    def v(ap, k):
        return ap.rearrange("p (k c) -> p k c", c=Cw)

    q4v, avwv, scv, ccv, GBv, ps2v = (v(x, k) for x, k in
                                      ((q4, 4), (avw, 3), (sc, 3), (cc, 3),
                                       (GB, 4), (ps2, 2)))

    # ---- temp allocator with per-width tag free lists ----
    free_tags = {}
    n_tags = [0]
    tag_of = {}

    def alloc(name, k=1, pool="dve"):
        fl = free_tags.setdefault((pool, k), [])
        if fl:
            tag = fl.pop()
        else:
            tag = f"{pool}w{k}_{n_tags[0]}"
            n_tags[0] += 1
        ap = pools[pool].tile([P, k * Cw], cdt, tag=tag, name=f"{name}_{ti}")
        tag_of[id(ap)] = (pool, tag, k)
        return ap

    def freet(*aps):
        for ap in aps:
            pool, tag, k = tag_of.pop(id(ap))
            free_tags[(pool, k)].append(tag)

    def bc(ap_pc, k):
        """broadcast a [P, Cw] AP across k components -> [P, k, Cw]"""
        return (ap_pc.rearrange("p (k c) -> p k c", k=1)
                .to_broadcast([P, k, Cw]))

    def bc4d(ap_pc):
        return (ap_pc.rearrange("p (a b c) -> p a b c", a=1, b=1)
                .to_broadcast([P, 2, 2, Cw]))

    # ---- stage 1: q_z* x q   (angle idx 2 = psi; pairs (W,Z),(X,Y)) ----
    mc = alloc("mc", 4)
    ms = alloc("ms", 4)
    mcv, msv = v(mc, 4), v(ms, 4)
    TT(mcv[:, :], bc(ccv[:, 2, :], 4), q4v[:, :], OP.mult)
    TT(msv[:, :], bc(scv[:, 2, :], 4), q4v[:, ::-1], OP.mult)
    t4 = alloc("t4", 4)
    t4v = v(t4, 4)
    TT(t4v[:, 0:2], mcv[:, 0:2], msv[:, 0:2], OP.add)
    TT(t4v[:, 2:4], mcv[:, 2:4], msv[:, 2:4], OP.subtract)

    # ---- stage 2: q_x* x t  (angle idx 0 = roll; swap within pairs) ----
    TT(mcv[:, :], bc(ccv[:, 0, :], 4), t4v[:, :], OP.mult)
    ms4d = ms.rearrange("p (a b c) -> p a b c", a=2, c=Cw)
    t4sw = t4.rearrange("p (a b c) -> p a b c", a=2, c=Cw)[:, :, ::-1]
    TT(ms4d, bc4d(scv[:, 0, :]), t4sw, OP.mult)
    freet(t4)
    u4 = alloc("u4", 4)      # aliases t4's slot (t4 fully consumed above)
    u4v = v(u4, 4)
    TT(u4v[:, 0:4:2], mcv[:, 0:4:2], msv[:, 0:4:2], OP.add)
    TT(u4v[:, 1:4:2], mcv[:, 1:4:2], msv[:, 1:4:2], OP.subtract)

    # ---- stage 3: q_y* x u  (angle idx 1 = pitch; rotate-2) ----
    TT(mcv[:, :], bc(ccv[:, 1, :], 4), u4v[:, :], OP.mult)
    ms4r = ms.rearrange("p (a b c) -> p a b c", b=2, c=Cw)
    u4rot = u4.rearrange("p (a b c) -> p a b c", b=2, c=Cw)[:, ::-1]
    TT(ms4r, bc4d(scv[:, 1, :]), u4rot, OP.mult)
    freet(u4)
    a4 = alloc("a4", 4)      # aliases u4's slot
    a4v = v(a4, 4)
    TT(a4v[:, 0:4:3], mcv[:, 0:4:3], msv[:, 0:4:3], OP.add)
    TT(a4v[:, 1:3], mcv[:, 1:3], msv[:, 1:3], OP.subtract)
    freet(mc, ms)

    # ---- products: Pab = (AB, AC), Pcd = (BD, CD), BBCC via ACT ----
    Pab = alloc("Pab", 2)
    Pabv = v(Pab, 2)
    TT(Pabv[:, :], bc(a4v[:, 0], 2), a4v[:, 1:3], OP.mult)
    Pcd = alloc("Pcd", 2)
    Pcdv = v(Pcd, 2)
    TT(Pcdv[:, :], a4v[:, 1:3], bc(a4v[:, 3], 2), OP.mult)
    if SQUARE_ON_ACT:
        bbcc = pools["mid"].tile([P, 2 * Cw], cdt, tag="bbcc",
                                 name=f"bbcc_{ti}")
        ACT(v(bbcc, 2), a4v[:, 1:3], AF.Square)
    else:
        bbcc = alloc("bbcc", 2)
        TT(v(bbcc, 2), a4v[:, 1:3], a4v[:, 1:3], OP.mult)
    if E13_ON_ACT:
        e13 = pools["mid"].tile([P, 2 * Cw], cdt, tag="e13", name=f"e13_{ti}")
        ACT(v(e13, 2), Pabv[:, :], AF.Copy, scale=fp["wa"])
    else:
        e13 = None
    freet(a4)

    # ---- M2 = (M02, M12) ----
    M2 = alloc("M2", 2)
    M2v = v(M2, 2)
    TT(M2v[:, 0], Pcdv[:, 0], Pabv[:, 1], OP.add)
    TT(M2v[:, 1], Pcdv[:, 1], Pabv[:, 0], OP.subtract)
    if E13_ON_ACT:
        freet(Pab, Pcd)   # e13 (ACT) was their last reader
    # s01 = (wr*psi) * M2
    s01 = alloc("s01", 2)
    TT(v(s01, 2)[:, :], bc(ps2v[:, 0, :], 2), M2v[:, :], OP.mult)
    freet(M2)
    # Sg = BB + CC ; s2 = (wr2*psi) * Sg
    Sg = alloc("Sg")
    TT(Sg[:], v(bbcc, 2)[:, 0], v(bbcc, 2)[:, 1], OP.add)
    s2 = alloc("s2")
    TT(s2[:], ps2v[:, 1, :], Sg[:], OP.mult)
    freet(Sg)

    # ---- G values into GB = (G3, G1, G0, G2) ----
    t01 = alloc("t01", 2)
    if e13 is None:
        e13l = alloc("e13", 2)
        nc.vector.tensor_scalar_mul(v(e13l, 2)[:, :], Pabv[:, :], fp["wa"])
        TT(v(t01, 2)[:, :], v(e13l, 2)[:, :], avwv[:, 0:2, :], OP.add)
        freet(e13l)
    else:
        TT(v(t01, 2)[:, :], v(e13, 2)[:, :], avwv[:, 0:2, :], OP.add)
    if not E13_ON_ACT:
        freet(Pab, Pcd)
    TT(GBv[:, 2:0:-1], v(t01, 2)[:, :], v(s01, 2)[:, :], OP.subtract)
    freet(t01, s01)
    g2a = alloc("g2a")
    TT(g2a[:], avwv[:, 2, :], ps2v[:, 1, :], OP.subtract)
    TT(GBv[:, 3], g2a[:], s2[:], OP.add)
    freet(g2a, s2)

    # ---- butterfly: UV = (U+, V+, U-, V-) ----
    UV = alloc("UV", 4)
    UVv = v(UV, 4)
    TT(UVv[:, 0:2], GBv[:, 0:2], GBv[:, 2:4], OP.add)
    TT(UVv[:, 2:4], GBv[:, 0:2], GBv[:, 2:4], OP.subtract)

    # ---- outs: out[r] = U_{sA[r]} + sB[r] * V_{sB[r]*sC[r]} ----
    out_t = pools["outp"].tile([P, 4 * Cw], cdt, tag="out", name=f"out_{ti}")
    out3 = v(out_t, 4)
    uidx = [0 if fp["sA"][r] > 0 else 2 for r in range(4)]
    vidx = [1 if fp["sB"][r] * fp["sC"][r] > 0 else 3 for r in range(4)]
    adds = [r for r in range(4) if fp["sB"][r] > 0]
    subs = [r for r in range(4) if fp["sB"][r] <= 0]

    def emit_outs(rset, op):
        # each computed group is DMA'd out immediately so the final
        # store overlaps the remaining Vector work
        while rset:
            if len(rset) >= 2:
                a, b = rset[0], rset[1]
                dst = out3[:, a : b + 1 : (b - a), :]

                def pair_ap(ia, ib):
                    if ib == ia:
                        return bc(UVv[:, ia], 2)
                    if ib > ia:
                        return UVv[:, ia : ib + 1 : ib - ia]
                    return UVv[:, ia :: ib - ia]

                TT(dst, pair_ap(uidx[a], uidx[b]), pair_ap(vidx[a], vidx[b]),
                   op)
                nc.sync.dma_start(
                    out=outT3[:, a : b + 1 : (b - a), c0 : c0 + Cw], in_=dst)
                rset = rset[2:]
            else:
                r = rset[0]
                TT(out3[:, r, :], UVv[:, uidx[r]], UVv[:, vidx[r]], op)
                nc.sync.dma_start(
                    out=outT3[:, r, c0 : c0 + Cw], in_=out3[:, r, :])
                rset = rset[1:]

    emit_outs(adds, OP.add)
    emit_outs(subs, OP.subtract)
    freet(UV)


# --------------------------------------------------------------------------
# Public entry point
# --------------------------------------------------------------------------
def kernel(root_state, control_target, mass, g, mixer, max_thrusts,
           gain_attitude, gain_angular_rate):
    root_state = np.asarray(root_state, np.float32)
    control_target = np.asarray(control_target, np.float32)
    assert root_state.shape == (B_TOTAL, 13), root_state.shape
    assert control_target.shape == (B_TOTAL, 4), control_target.shape

    fp = _fold_params(mass, g, mixer, max_thrusts, gain_attitude, gain_angular_rate)

    key = hashlib.sha256(
        repr(({k: v for k, v in fp.items() if k != "Wf"}, COMPUTE_DT,
              tuple(TILE_WIDTHS), SQUARE_ON_ACT, E13_ON_ACT)).encode()
    ).hexdigest()
    if key not in _CACHE:
        _install_bir_patch()
        _CACHE[key] = _build_nc(fp)
    nc = _CACHE[key]

    from concourse.bass_utils import run_bass_kernel_spmd

    in_maps = make_in_maps(root_state, control_target)
    res = run_bass_kernel_spmd(nc, in_maps, core_ids=list(range(N_CORES)))
    return assemble_out([res.results[i]["outT"] for i in range(N_CORES)])
